# revision 34
# baseline (speedup 1.0000x reference)
"""DeepseekV3 MoE layer on 8 Trainium2 NeuronCores.

Strategy (expert-parallel, per sharding hint):
- Host does the routing (gate scores, top-4, combine weights) and the
  all-to-all token dispatch as input sharding: each core receives its 2
  experts' gathered tokens pre-transposed to [H, C] fp16. Each core's
  LARGER expert goes in slot 0 so slot 1 compiles with a smaller padded
  token count (less PE waste).
- The gathered->dense combine is a MATMUL against host-built selection
  matrices (S-tiles) with the combine weights folded in: for each output
  token-tile, one PSUM accumulation group sums the shared-expert down
  projection and both experts' contributions (S^T @ z). No indirect
  DMAs / scatter-adds anywhere.
- All large inputs are host-packed so each DMA is 128 long contiguous
  descriptors (HWDGE issue cost scales with descriptor count).
- A tiny dummy collective at t~0 absorbs the ~11.5us first-collective
  entry cost; NHALF chunked ReduceScatters then overlap compute.
"""

import os
import sys
import types

sys.path.insert(0, "/opt/trn_rl_repo")

# antenv.axon_hooks shim so trace=True works under axon (profiling only).
if "antenv.axon_hooks" not in sys.modules:
    _hook_holder = [None]
    _hooks_mod = types.ModuleType("antenv.axon_hooks")
    _hooks_mod.set_axon_ntff_profile_hook = lambda h: _hook_holder.__setitem__(0, h)
    _hooks_mod.get_axon_ntff_profile_hook = lambda: _hook_holder[0]
    sys.modules["antenv.axon_hooks"] = _hooks_mod
    try:
        from trn_agent_boot.trn_boot import _ntff_profile_via_ctypes

        _hook_holder[0] = _ntff_profile_via_ctypes("/opt/axon/libaxon_pjrt.so")
    except Exception:
        pass

import numpy as np

import concourse.mybir as mybir
from concourse import bacc
from concourse.tile import TileContext, add_dep_helper
from concourse.bass_utils import run_bass_kernel_spmd

N_CORES = 8
T, H, E, I = 2048, 1024, 16, 512
TOPK = 4
SIC = 128  # shared-expert intermediate slice per core (1024 / 8)
EPC = 2  # experts per core
OOB = 1 << 20
NTC = T // 128  # output token tiles
NQ = 4  # y_acc quarters (4 tiles each)
# RS chunks as quarter ranges: big early chunk, small final chunks
CHUNKS = [(0, 2), (2, 3), (3, 4)]
CROWS = [(q1 - q0) * T // NQ // N_CORES for q0, q1 in CHUNKS]  # [128, 64, 64]
COFF = [sum(CROWS[:i]) for i in range(len(CHUNKS))]

F16 = mybir.dt.float16
F32 = mybir.dt.float32
AF = mybir.ActivationFunctionType

_nc_cache = {}
last_exec_time_ns = None


def _segs(c_use):
    out = []
    s0 = 0
    while s0 < c_use:
        s1 = min(s0 + 512, c_use)
        out.append((s0, s1))
        s0 = s1
    return out


def _build(Cu, Cp, ovl):
    """Cu/Cp: per-slot (use, pad) token counts. ovl: tuple over
    j=(k*NCC0+cc) of (tc_lo, tc_hi) token-tile range (union over cores),
    None for absent chunks."""
    NCC = [Cp[0] // 128, Cp[1] // 128]
    NCC0 = NCC[0]
    smap = {}
    NS = 0
    for j, r in enumerate(ovl):
        if r is None:
            continue
        for tcv in range(r[0], r[1] + 1):
            smap[(j, tcv)] = NS
            NS += 1
    contrib = [[] for _ in range(NTC)]
    for (j, tcv), n in smap.items():
        contrib[tcv].append((n, j))
    for lst in contrib:
        lst.sort()

    nc = bacc.Bacc(trn_type="TRN2", target_bir_lowering=False, num_devices=N_CORES)

    # ---- I/O (host-packed for contiguous per-partition DMA) ----
    xTp = nc.dram_tensor("xTp", [4, 128, H // 128, T // 4], F16, kind="ExternalInput")
    xgT16 = nc.dram_tensor("xgT16", [EPC, NCC0, 128, H // 128, 128], F16, kind="ExternalInput")
    wgup = nc.dram_tensor("wgup", [EPC, 128, H // 128, 2 * I], F16, kind="ExternalInput")
    wdp = nc.dram_tensor("wdp", [EPC, 128, I // 128, H], F16, kind="ExternalInput")
    sgsup = nc.dram_tensor("sgsup", [128, H // 128, 2 * SIC], F16, kind="ExternalInput")
    sd16 = nc.dram_tensor("sd16", [SIC, H], F16, kind="ExternalInput")
    Sp = nc.dram_tensor("Sp", [128, max(NS, 1), 128], F16, kind="ExternalInput")

    # y_acc in quarter-major / partition-major layout: token t lives at
    # [t // 512, t % 128, (t % 512) // 128, :] so the ys->y_acc DMA is one
    # contiguous run per partition. RS shards by flat order — host
    # reassembles per chunk.
    y_acc = nc.dram_tensor("y_acc", [NQ, 128, NTC // NQ, H], F16)
    rs_b = nc.dram_tensor("rs_b", [T // N_CORES, H], F16)
    y_out = nc.dram_tensor("y_out", [T // N_CORES, H], F16, kind="ExternalOutput")
    warm_i = nc.dram_tensor("warm_i", [N_CORES, 64], F16)
    warm_o = nc.dram_tensor("warm_o", [1, 64], F16)

    with TileContext(nc) as tc:
        with (
            tc.tile_pool(name="res", bufs=1) as res,
            tc.tile_pool(name="xtq", bufs=2) as xtp_pool,
            tc.tile_pool(name="sc", bufs=3) as scp,
            tc.tile_pool(name="ps_gu", bufs=2, space="PSUM") as ps_gu,
            tc.tile_pool(name="ps_z", bufs=2, space="PSUM") as ps_z,
        ):
            # ---- resident tiles ----
            xgT_sb = res.tile([128, EPC, H // 128, Cp[0]], F16, tag="xgT")
            wgu_sb = res.tile([128, EPC, H // 128, 2 * I], F16, tag="wgu")
            wd_sb = res.tile([128, EPC, I // 128, H], F16, tag="wd")
            sgsu_sb = res.tile([128, H // 128, 2 * SIC], F16, tag="sgsu")
            sd_sb = res.tile([128, H], F16, tag="sd")
            S_sb = res.tile([128, max(NS, 1), 128], F16, tag="S")
            spT_sb = res.tile([128, T], F16, tag="spT")
            p_sb = res.tile([128, EPC, I // 128, Cp[0]], F16, tag="p")
            z_sb = res.tile([128, EPC, NCC0, H], F16, tag="z")
            ys_sb = res.tile([128, NTC, H], F16, tag="ys")

            xt_tiles = []

            # warm up the collectives engine (absorbs first-CC entry cost)
            nc.gpsimd.collective_compute(
                "ReduceScatter", mybir.AluOpType.add,
                replica_groups=[list(range(N_CORES))],
                ins=[warm_i.ap().opt()], outs=[warm_o.ap().opt()])

            # ---- preload ----
            # scalar gets sgsu (small, needed first) so it lands while
            # sync streams the first xT quarter (split for earlier start)
            nc.scalar.dma_start(sgsu_sb[:], sgsup.ap())
            for q in range(4):
                xtq = xtp_pool.tile([128, H // 128, T // 4], F16, tag="xtq")
                if q == 0:
                    nc.sync.dma_start(xtq[:, 0:4, :], xTp.ap()[0, :, 0:4, :])
                    nc.sync.dma_start(xtq[:, 4:8, :], xTp.ap()[0, :, 4:8, :])
                else:
                    nc.sync.dma_start(xtq[:], xTp.ap()[q])
                xt_tiles.append(xtq)
            # scalar (HWDGE): sd + packed expert weights, consumption order
            nc.scalar.dma_start(sd_sb[:], sd16.ap())
            for e in range(EPC):
                nc.scalar.dma_start(wgu_sb[:, e], wgup.ap()[e])
            for e in range(EPC):
                nc.scalar.dma_start(wd_sb[:, e], wdp.ap()[e])
            # gpsimd (SWDGE): gathered tokens + S-tiles
            for e in range(EPC):
                for cc in range(NCC[e]):
                    nc.gpsimd.dma_start(
                        xgT_sb[:, e, :, cc * 128:(cc + 1) * 128], xgT16.ap()[e, cc])
            nc.gpsimd.dma_start(S_sb[:], Sp.ap())

            # zero the pad columns of p (read by down-matmul lhsT chunks)
            for e in range(EPC):
                if Cp[e] > Cu[e]:
                    nc.vector.memset(p_sb[:, e, :, Cu[e]:Cp[e]], 0)

            # ---- emit helpers ----
            def emit_shared_gu(s):
                pg = ps_gu.tile([128, 512], F32, tag="pg")
                pu = ps_gu.tile([128, 512], F32, tag="pu")
                for ho in range(H // 128):
                    nc.tensor.matmul(
                        pg[:], lhsT=sgsu_sb[:, ho, 0:SIC], rhs=xt_tiles[s][:, ho, :],
                        start=(ho == 0), stop=(ho == H // 128 - 1))
                    nc.tensor.matmul(
                        pu[:], lhsT=sgsu_sb[:, ho, SIC:2 * SIC], rhs=xt_tiles[s][:, ho, :],
                        start=(ho == 0), stop=(ho == H // 128 - 1))
                sg = scp.tile([128, 512], F16, tag="sg")
                nc.scalar.activation(sg[:], pg[:], AF.Silu)
                nc.vector.tensor_tensor(
                    out=spT_sb[:, s * 512:(s + 1) * 512], in0=sg[:], in1=pu[:],
                    op=mybir.AluOpType.mult)

            def emit_expert_gu(e, seg):
                a, b = seg
                w = b - a
                for it in range(I // 128):
                    pg_full = ps_gu.tile([128, 512], F32, tag="pg")
                    pg = pg_full[:, :w]
                    pu_full = ps_gu.tile([128, 512], F32, tag="pu")
                    pu = pu_full[:, :w]
                    for ho in range(H // 128):
                        nc.tensor.matmul(
                            pg[:], lhsT=wgu_sb[:, e, ho, it * 128:(it + 1) * 128],
                            rhs=xgT_sb[:, e, ho, a:b],
                            start=(ho == 0), stop=(ho == H // 128 - 1))
                        nc.tensor.matmul(
                            pu[:], lhsT=wgu_sb[:, e, ho, I + it * 128:I + (it + 1) * 128],
                            rhs=xgT_sb[:, e, ho, a:b],
                            start=(ho == 0), stop=(ho == H // 128 - 1))
                    sg_full = scp.tile([128, 512], F16, tag="sg")
                    sg = sg_full[:, :w]
                    nc.scalar.activation(sg[:], pg[:], AF.Silu)
                    nc.vector.tensor_tensor(
                        out=p_sb[:, e, it, a:b], in0=sg[:], in1=pu[:],
                        op=mybir.AluOpType.mult)

            def emit_down(e, cc):
                pz = ps_z.tile([128, H], F32, tag="pz")
                for it in range(I // 128):
                    for hf in range(2):
                        nc.tensor.matmul(
                            pz[:, hf * 512:(hf + 1) * 512],
                            lhsT=p_sb[:, e, it, cc * 128:(cc + 1) * 128],
                            rhs=wd_sb[:, e, it, hf * 512:(hf + 1) * 512],
                            start=(it == 0), stop=(it == I // 128 - 1))
                nc.vector.tensor_copy(z_sb[:, e, cc, :], pz[:])

            def emit_group(tc_i):
                """shared down + S-combine for output token tile tc_i.
                Reuses the gu-phase PSUM banks (pg/pu tags)."""
                py0 = ps_gu.tile([128, 512], F32, tag="pg")
                py1 = ps_gu.tile([128, 512], F32, tag="pu")
                nmm = len(contrib[tc_i]) + 1
                for hf, py in enumerate((py0, py1)):
                    nc.tensor.matmul(
                        py[:],
                        lhsT=spT_sb[:, tc_i * 128:(tc_i + 1) * 128],
                        rhs=sd_sb[:, hf * 512:(hf + 1) * 512],
                        start=True, stop=(nmm == 1))
                    for i, (n, j) in enumerate(contrib[tc_i]):
                        e, cc = j // NCC0, j % NCC0
                        nc.tensor.matmul(
                            py[:],
                            lhsT=S_sb[:, n, :],
                            rhs=z_sb[:, e, cc, hf * 512:(hf + 1) * 512],
                            start=False, stop=(i == nmm - 2))
                    if hf == 0:
                        nc.scalar.activation(
                            ys_sb[:, tc_i, 0:512], py[:], AF.Copy)
                    else:
                        nc.vector.tensor_copy(ys_sb[:, tc_i, 512:1024], py[:])

            WG = NTC // NQ  # output tiles per y_acc write (one quarter)
            yacc_wr = [None] * NQ
            rs_insts = [None] * len(CHUNKS)

            def emit_yacc_write(wi):
                yacc_wr[wi] = nc.sync.dma_start(
                    y_acc.ap()[wi],
                    ys_sb[:, wi * WG:(wi + 1) * WG, :])

            def emit_rs(ci):
                qa, qb = CHUNKS[ci]
                cc_inst = nc.gpsimd.collective_compute(
                    "ReduceScatter",
                    mybir.AluOpType.add,
                    replica_groups=[list(range(N_CORES))],
                    ins=[y_acc.ap()[qa:qb].opt()],
                    outs=[rs_b.ap()[COFF[ci]:COFF[ci] + CROWS[ci], :].opt()],
                )
                for wi in range(qa, qb):
                    add_dep_helper(cc_inst.ins, yacc_wr[wi].ins,
                                   reason="rs after y_acc init")
                rs_insts[ci] = cc_inst

            # ---- emission schedule ----
            # Downs for seg0-covered chunks run right after each expert's
            # seg0 gu; the small seg1 gu and last chunks come after the
            # first-half groups so RS0 can fire early.
            seg0 = _segs(Cu[0])
            seg1 = _segs(Cu[1])
            done_j = set(j for j, r in enumerate(ovl) if r is None)
            next_tc = 0
            n_sh = 0

            rs_bounds = {CHUNKS[ci][1] * WG: ci for ci in range(len(CHUNKS))}

            def flush_groups():
                nonlocal next_tc
                while (next_tc < NTC
                       and next_tc * 128 < n_sh * 512
                       and all(j in done_j for _, j in contrib[next_tc])):
                    emit_group(next_tc)
                    next_tc += 1
                    if next_tc % WG == 0:
                        emit_yacc_write(next_tc // WG - 1)
                    if next_tc in rs_bounds:
                        emit_rs(rs_bounds[next_tc])

            # First-half groups (tc < 8) only need shared segs 0-1 and
            # seg0-covered chunks, so RS0 fires before sh2/sh3/seg1 run.
            early = [min(4, NCC[e]) for e in range(EPC)]
            emit_shared_gu(0)
            emit_shared_gu(1)
            n_sh = 2
            emit_expert_gu(0, seg0[0])
            for cc in range(early[0]):
                emit_down(0, cc)
                done_j.add(cc)
            emit_expert_gu(1, seg1[0])
            for cc in range(early[1]):
                emit_down(1, cc)
                done_j.add(NCC0 + cc)
            flush_groups()
            emit_shared_gu(2)
            n_sh = 3
            emit_shared_gu(3)
            n_sh = 4
            flush_groups()
            for s in seg0[1:]:
                emit_expert_gu(0, s)
            for cc in range(early[0], NCC[0]):
                emit_down(0, cc)
                done_j.add(cc)
            flush_groups()
            for s in seg1[1:]:
                emit_expert_gu(1, s)
            for cc in range(early[1], NCC[1]):
                emit_down(1, cc)
                done_j.add(NCC0 + cc)
            flush_groups()
            assert next_tc == NTC, f"groups not all emitted: {next_tc}"

            # DRAM->DRAM copy of the RS shards to the kernel output
            for ci in range(len(CHUNKS)):
                cp = nc.sync.dma_start(
                    y_out.ap()[COFF[ci]:COFF[ci] + CROWS[ci], :],
                    rs_b.ap()[COFF[ci]:COFF[ci] + CROWS[ci], :])
                add_dep_helper(cp.ins, rs_insts[ci].ins, reason="copy rs output")

    nc.compile()
    return nc


def _get_nc(Cu, Cp, ovl):
    key = (Cu, Cp, ovl)
    if key not in _nc_cache:
        _nc_cache[key] = _build(Cu, Cp, ovl)
    return _nc_cache[key]


def kernel(hidden_states, gate_w, expert_gate, expert_up, expert_down,
           shared_gate, shared_up, shared_down):
    global last_exec_time_ns
    B, S, Hh = hidden_states.shape
    x = np.asarray(hidden_states, np.float32).reshape(-1, Hh)

    # ---- host-side routing (the MoE gate) ----
    gw = np.asarray(gate_w, np.float32)
    logits = x @ gw.T
    scores = 1.0 / (1.0 + np.exp(-logits))
    order = np.argsort(-scores, axis=1, kind="stable")[:, :TOPK]
    topk_w = np.take_along_axis(scores, order, axis=1)
    topk_w = topk_w / (topk_w.sum(-1, keepdims=True) + 1e-20)
    Wc = np.zeros((T, E), np.float32)  # dense combine matrix
    np.add.at(Wc, (np.arange(T)[:, None], order), topk_w)
    sel = Wc > 0
    counts = sel.sum(0)

    # slot assignment: each core's larger expert -> slot 0
    slot_exp = []  # per core: (e_slot0, e_slot1)
    for c in range(N_CORES):
        e0, e1 = EPC * c, EPC * c + 1
        if counts[e1] > counts[e0]:
            e0, e1 = e1, e0
        slot_exp.append((e0, e1))
    Cu, Cp = [], []
    for k in range(EPC):
        m = max(int(counts[slot_exp[c][k]]) for c in range(N_CORES))
        cu = min(max(64, -(-m // 64) * 64), T)
        Cu.append(cu)
        Cp.append(-(-cu // 128) * 128)
    Cu, Cp = tuple(Cu), tuple(Cp)
    NCC = [Cp[0] // 128, Cp[1] // 128]
    NCC0 = NCC[0]

    gidx_all = np.zeros((E, Cp[0]), np.int32)
    sidx_all = np.full((E, Cp[0]), OOB, np.int32)
    for e in range(E):
        lst = np.nonzero(sel[:, e])[0].astype(np.int32)
        gidx_all[e, :len(lst)] = lst
        sidx_all[e, :len(lst)] = lst

    # ---- overlap structure: token-tile range per (slot, chunk), union ----
    ovl = []
    for k in range(EPC):
        for cc in range(NCC0):
            lo, hi = NTC, -1
            if cc < NCC[k]:
                for c in range(N_CORES):
                    e = slot_exp[c][k]
                    r = sidx_all[e, cc * 128:(cc + 1) * 128]
                    r = r[r < OOB]
                    if len(r):
                        lo = min(lo, int(r.min()) // 128)
                        hi = max(hi, int(r.max()) // 128)
            ovl.append(None if hi < 0 else (lo, hi))
    ovl = tuple(ovl)
    smap = {}
    NS = 0
    for j, r in enumerate(ovl):
        if r is None:
            continue
        for tcv in range(r[0], r[1] + 1):
            smap[(j, tcv)] = NS
            NS += 1

    # ---- cast / pack per-core inputs (the all-to-all token dispatch) ----
    x16 = x.astype(np.float16)
    xTp = np.ascontiguousarray(
        x16.reshape(4, T // 4, H // 128, 128).transpose(0, 3, 2, 1))
    eg = np.asarray(expert_gate, np.float32).astype(np.float16)
    eu = np.asarray(expert_up, np.float32).astype(np.float16)
    ed = np.asarray(expert_down, np.float32).astype(np.float16)
    sg = np.asarray(shared_gate, np.float32).astype(np.float16)
    su = np.asarray(shared_up, np.float32).astype(np.float16)
    sd = np.asarray(shared_down, np.float32).astype(np.float16)

    in_maps = []
    for c in range(N_CORES):
        ex = slot_exp[c]
        xgT = np.stack([
            np.ascontiguousarray(
                x16[gidx_all[e]].T.reshape(H // 128, 128, NCC0, 128)
                .transpose(2, 1, 0, 3))
            for e in ex
        ])
        wgu = np.stack([
            np.concatenate([eg[e], eu[e]], axis=1)
            .reshape(H // 128, 128, 2 * I).transpose(1, 0, 2)
            for e in ex
        ])
        wd = np.stack([
            ed[e].reshape(I // 128, 128, H).transpose(1, 0, 2)
            for e in ex
        ])
        sgsu = np.concatenate([sg[:, c * SIC:(c + 1) * SIC],
                               su[:, c * SIC:(c + 1) * SIC]], axis=1)
        sgsup = sgsu.reshape(H // 128, 128, 2 * SIC).transpose(1, 0, 2)
        # S-tiles: selection matrices with combine weights folded in
        Sp = np.zeros((128, max(NS, 1), 128), np.float16)
        for k, e in enumerate(ex):
            for cc in range(NCC[k]):
                j = k * NCC0 + cc
                if ovl[j] is None:
                    continue
                toks = sidx_all[e, cc * 128:(cc + 1) * 128]
                valid = toks < OOB
                wv = Wc[gidx_all[e, cc * 128:(cc + 1) * 128], e] * valid
                for tcv in range(ovl[j][0], ovl[j][1] + 1):
                    n = smap[(j, tcv)]
                    m = valid & (toks // 128 == tcv)
                    pp = np.nonzero(m)[0]
                    Sp[pp, n, toks[m] % 128] = wv[pp].astype(np.float16)
        in_maps.append({
            "xTp": xTp,
            "xgT16": xgT,
            "wgup": np.ascontiguousarray(wgu),
            "wdp": np.ascontiguousarray(wd),
            "sgsup": np.ascontiguousarray(sgsup),
            "sd16": np.ascontiguousarray(sd[c * SIC:(c + 1) * SIC, :]),
            "Sp": Sp,
        })

    nc = _get_nc(Cu, Cp, ovl)
    trace = bool(int(os.environ.get("KERNEL_TRACE", "0")))
    res = run_bass_kernel_spmd(
        nc, in_maps, core_ids=list(range(N_CORES)), trace=trace
    )
    last_exec_time_ns = res.exec_time_ns

    # reassemble: RS chunk ci covers quarters [qa, qb); shard c is the
    # flat 1/8 slice: row r -> A = c*rows + r, tci = A % 4, B = A // 4,
    # q = qa + B // 128, p = B % 128, token = q*512 + tci*128 + p
    out = np.empty((T, Hh), np.float32)
    for c in range(N_CORES):
        yo = np.asarray(res.results[c]["y_out"], np.float32)
        for ci, (qa, qb) in enumerate(CHUNKS):
            rows = CROWS[ci]
            A = c * rows + np.arange(rows)
            tci = A % 4
            Bq = A // 4
            tok = (qa + Bq // 128) * 512 + tci * 128 + (Bq % 128)
            out[tok] = yo[COFF[ci]:COFF[ci] + rows]
    return out.reshape(B, S, Hh).astype(np.float32)


# revision 35
# speedup vs baseline: 1.0076x; 1.0076x over previous
"""DeepseekV3 MoE layer on 8 Trainium2 NeuronCores.

Strategy (expert-parallel, per sharding hint):
- Host does the routing (gate scores, top-4, combine weights) and the
  all-to-all token dispatch as input sharding: each core receives its 2
  experts' gathered tokens pre-transposed to [H, C] fp16. Each core's
  LARGER expert goes in slot 0 so slot 1 compiles with a smaller padded
  token count (less PE waste).
- The gathered->dense combine is a MATMUL against host-built selection
  matrices (S-tiles) with the combine weights folded in: for each output
  token-tile, one PSUM accumulation group sums the shared-expert down
  projection and both experts' contributions (S^T @ z). No indirect
  DMAs / scatter-adds anywhere.
- All large inputs are host-packed so each DMA is 128 long contiguous
  descriptors (HWDGE issue cost scales with descriptor count).
- A tiny dummy collective at t~0 absorbs the ~11.5us first-collective
  entry cost; NHALF chunked ReduceScatters then overlap compute.
"""

import os
import sys
import types

sys.path.insert(0, "/opt/trn_rl_repo")

# antenv.axon_hooks shim so trace=True works under axon (profiling only).
if "antenv.axon_hooks" not in sys.modules:
    _hook_holder = [None]
    _hooks_mod = types.ModuleType("antenv.axon_hooks")
    _hooks_mod.set_axon_ntff_profile_hook = lambda h: _hook_holder.__setitem__(0, h)
    _hooks_mod.get_axon_ntff_profile_hook = lambda: _hook_holder[0]
    sys.modules["antenv.axon_hooks"] = _hooks_mod
    try:
        from trn_agent_boot.trn_boot import _ntff_profile_via_ctypes

        _hook_holder[0] = _ntff_profile_via_ctypes("/opt/axon/libaxon_pjrt.so")
    except Exception:
        pass

import numpy as np

import concourse.mybir as mybir
from concourse import bacc
from concourse.tile import TileContext, add_dep_helper
from concourse.bass_utils import run_bass_kernel_spmd

N_CORES = 8
T, H, E, I = 2048, 1024, 16, 512
TOPK = 4
SIC = 128  # shared-expert intermediate slice per core (1024 / 8)
EPC = 2  # experts per core
OOB = 1 << 20
NTC = T // 128  # output token tiles
NQ = 4  # y_acc quarters (4 tiles each)
# RS chunks as quarter ranges (2MB chunks; smaller ones run below
# 60 GB/s and lose more than the extra pipelining gains)
CHUNKS = [(0, 2), (2, 4)]
CROWS = [(q1 - q0) * T // NQ // N_CORES for q0, q1 in CHUNKS]  # [128, 64, 64]
COFF = [sum(CROWS[:i]) for i in range(len(CHUNKS))]

F16 = mybir.dt.float16
F32 = mybir.dt.float32
AF = mybir.ActivationFunctionType

_nc_cache = {}
last_exec_time_ns = None


def _segs(c_use):
    out = []
    s0 = 0
    while s0 < c_use:
        s1 = min(s0 + 512, c_use)
        out.append((s0, s1))
        s0 = s1
    return out


def _build(Cu, Cp, ovl):
    """Cu/Cp: per-slot (use, pad) token counts. ovl: tuple over
    j=(k*NCC0+cc) of (tc_lo, tc_hi) token-tile range (union over cores),
    None for absent chunks."""
    NCC = [Cp[0] // 128, Cp[1] // 128]
    NCC0 = NCC[0]
    smap = {}
    NS = 0
    for j, r in enumerate(ovl):
        if r is None:
            continue
        for tcv in range(r[0], r[1] + 1):
            smap[(j, tcv)] = NS
            NS += 1
    contrib = [[] for _ in range(NTC)]
    for (j, tcv), n in smap.items():
        contrib[tcv].append((n, j))
    for lst in contrib:
        lst.sort()

    nc = bacc.Bacc(trn_type="TRN2", target_bir_lowering=False, num_devices=N_CORES)

    # ---- I/O (host-packed for contiguous per-partition DMA) ----
    xTp = nc.dram_tensor("xTp", [4, 128, H // 128, T // 4], F16, kind="ExternalInput")
    xgT16 = nc.dram_tensor("xgT16", [EPC, NCC0, 128, H // 128, 128], F16, kind="ExternalInput")
    wgup = nc.dram_tensor("wgup", [EPC, 128, H // 128, 2 * I], F16, kind="ExternalInput")
    wdp = nc.dram_tensor("wdp", [EPC, 128, I // 128, H], F16, kind="ExternalInput")
    sgsup = nc.dram_tensor("sgsup", [128, H // 128, 2 * SIC], F16, kind="ExternalInput")
    sd16 = nc.dram_tensor("sd16", [SIC, H], F16, kind="ExternalInput")
    Sp = nc.dram_tensor("Sp", [128, max(NS, 1), 128], F16, kind="ExternalInput")

    # y_acc in quarter-major / partition-major layout: token t lives at
    # [t // 512, t % 128, (t % 512) // 128, :] so the ys->y_acc DMA is one
    # contiguous run per partition. RS shards by flat order — host
    # reassembles per chunk.
    y_acc = nc.dram_tensor("y_acc", [NQ, 128, NTC // NQ, H], F16)
    rs_b = nc.dram_tensor("rs_b", [T // N_CORES, H], F16)
    y_out = nc.dram_tensor("y_out", [T // N_CORES, H], F16, kind="ExternalOutput")
    warm_i = nc.dram_tensor("warm_i", [N_CORES, 64], F16)
    warm_o = nc.dram_tensor("warm_o", [1, 64], F16)

    with TileContext(nc) as tc:
        with (
            tc.tile_pool(name="res", bufs=1) as res,
            tc.tile_pool(name="xtq", bufs=2) as xtp_pool,
            tc.tile_pool(name="sc", bufs=3) as scp,
            tc.tile_pool(name="ps_gu", bufs=2, space="PSUM") as ps_gu,
            tc.tile_pool(name="ps_z", bufs=2, space="PSUM") as ps_z,
        ):
            # ---- resident tiles ----
            xgT_sb = res.tile([128, EPC, H // 128, Cp[0]], F16, tag="xgT")
            wgu_sb = res.tile([128, EPC, H // 128, 2 * I], F16, tag="wgu")
            wd_sb = res.tile([128, EPC, I // 128, H], F16, tag="wd")
            sgsu_sb = res.tile([128, H // 128, 2 * SIC], F16, tag="sgsu")
            sd_sb = res.tile([128, H], F16, tag="sd")
            S_sb = res.tile([128, max(NS, 1), 128], F16, tag="S")
            spT_sb = res.tile([128, T], F16, tag="spT")
            p_sb = res.tile([128, EPC, I // 128, Cp[0]], F16, tag="p")
            z_sb = res.tile([128, EPC, NCC0, H], F16, tag="z")
            ys_sb = res.tile([128, NTC, H], F16, tag="ys")

            xt_tiles = []

            # warm up the collectives engine (absorbs first-CC entry cost)
            nc.gpsimd.collective_compute(
                "ReduceScatter", mybir.AluOpType.add,
                replica_groups=[list(range(N_CORES))],
                ins=[warm_i.ap().opt()], outs=[warm_o.ap().opt()])

            # ---- preload ----
            # scalar gets sgsu (small, needed first) so it lands while
            # sync streams the first xT quarter (split for earlier start)
            nc.scalar.dma_start(sgsu_sb[:], sgsup.ap())
            for q in range(4):
                xtq = xtp_pool.tile([128, H // 128, T // 4], F16, tag="xtq")
                if q == 0:
                    nc.sync.dma_start(xtq[:, 0:4, :], xTp.ap()[0, :, 0:4, :])
                    nc.sync.dma_start(xtq[:, 4:8, :], xTp.ap()[0, :, 4:8, :])
                else:
                    nc.sync.dma_start(xtq[:], xTp.ap()[q])
                xt_tiles.append(xtq)
            # scalar (HWDGE): sd + packed expert weights, consumption order
            nc.scalar.dma_start(sd_sb[:], sd16.ap())
            for e in range(EPC):
                nc.scalar.dma_start(wgu_sb[:, e], wgup.ap()[e])
            for e in range(EPC):
                nc.scalar.dma_start(wd_sb[:, e], wdp.ap()[e])
            # gpsimd (SWDGE): gathered tokens + S-tiles
            for e in range(EPC):
                for cc in range(NCC[e]):
                    nc.gpsimd.dma_start(
                        xgT_sb[:, e, :, cc * 128:(cc + 1) * 128], xgT16.ap()[e, cc])
            nc.gpsimd.dma_start(S_sb[:], Sp.ap())

            # zero the pad columns of p (read by down-matmul lhsT chunks)
            for e in range(EPC):
                if Cp[e] > Cu[e]:
                    nc.vector.memset(p_sb[:, e, :, Cu[e]:Cp[e]], 0)

            # ---- emit helpers ----
            def emit_shared_gu(s):
                pg = ps_gu.tile([128, 512], F32, tag="pg")
                pu = ps_gu.tile([128, 512], F32, tag="pu")
                for ho in range(H // 128):
                    nc.tensor.matmul(
                        pg[:], lhsT=sgsu_sb[:, ho, 0:SIC], rhs=xt_tiles[s][:, ho, :],
                        start=(ho == 0), stop=(ho == H // 128 - 1))
                    nc.tensor.matmul(
                        pu[:], lhsT=sgsu_sb[:, ho, SIC:2 * SIC], rhs=xt_tiles[s][:, ho, :],
                        start=(ho == 0), stop=(ho == H // 128 - 1))
                sg = scp.tile([128, 512], F16, tag="sg")
                nc.scalar.activation(sg[:], pg[:], AF.Silu)
                nc.vector.tensor_tensor(
                    out=spT_sb[:, s * 512:(s + 1) * 512], in0=sg[:], in1=pu[:],
                    op=mybir.AluOpType.mult)

            def emit_expert_gu(e, seg):
                a, b = seg
                w = b - a
                for it in range(I // 128):
                    pg_full = ps_gu.tile([128, 512], F32, tag="pg")
                    pg = pg_full[:, :w]
                    pu_full = ps_gu.tile([128, 512], F32, tag="pu")
                    pu = pu_full[:, :w]
                    for ho in range(H // 128):
                        nc.tensor.matmul(
                            pg[:], lhsT=wgu_sb[:, e, ho, it * 128:(it + 1) * 128],
                            rhs=xgT_sb[:, e, ho, a:b],
                            start=(ho == 0), stop=(ho == H // 128 - 1))
                        nc.tensor.matmul(
                            pu[:], lhsT=wgu_sb[:, e, ho, I + it * 128:I + (it + 1) * 128],
                            rhs=xgT_sb[:, e, ho, a:b],
                            start=(ho == 0), stop=(ho == H // 128 - 1))
                    sg_full = scp.tile([128, 512], F16, tag="sg")
                    sg = sg_full[:, :w]
                    nc.scalar.activation(sg[:], pg[:], AF.Silu)
                    nc.vector.tensor_tensor(
                        out=p_sb[:, e, it, a:b], in0=sg[:], in1=pu[:],
                        op=mybir.AluOpType.mult)

            def emit_down(e, cc):
                pz = ps_z.tile([128, H], F32, tag="pz")
                for it in range(I // 128):
                    for hf in range(2):
                        nc.tensor.matmul(
                            pz[:, hf * 512:(hf + 1) * 512],
                            lhsT=p_sb[:, e, it, cc * 128:(cc + 1) * 128],
                            rhs=wd_sb[:, e, it, hf * 512:(hf + 1) * 512],
                            start=(it == 0), stop=(it == I // 128 - 1))
                nc.vector.tensor_copy(z_sb[:, e, cc, :], pz[:])

            def emit_group(tc_i):
                """shared down + S-combine for output token tile tc_i.
                Reuses the gu-phase PSUM banks (pg/pu tags)."""
                py0 = ps_gu.tile([128, 512], F32, tag="pg")
                py1 = ps_gu.tile([128, 512], F32, tag="pu")
                nmm = len(contrib[tc_i]) + 1
                for hf, py in enumerate((py0, py1)):
                    nc.tensor.matmul(
                        py[:],
                        lhsT=spT_sb[:, tc_i * 128:(tc_i + 1) * 128],
                        rhs=sd_sb[:, hf * 512:(hf + 1) * 512],
                        start=True, stop=(nmm == 1))
                    for i, (n, j) in enumerate(contrib[tc_i]):
                        e, cc = j // NCC0, j % NCC0
                        nc.tensor.matmul(
                            py[:],
                            lhsT=S_sb[:, n, :],
                            rhs=z_sb[:, e, cc, hf * 512:(hf + 1) * 512],
                            start=False, stop=(i == nmm - 2))
                    if hf == 0:
                        nc.scalar.activation(
                            ys_sb[:, tc_i, 0:512], py[:], AF.Copy)
                    else:
                        nc.vector.tensor_copy(ys_sb[:, tc_i, 512:1024], py[:])

            WG = NTC // NQ  # output tiles per y_acc write (one quarter)
            yacc_wr = [None] * NQ
            rs_insts = [None] * len(CHUNKS)

            def emit_yacc_write(wi):
                yacc_wr[wi] = nc.sync.dma_start(
                    y_acc.ap()[wi],
                    ys_sb[:, wi * WG:(wi + 1) * WG, :])

            def emit_rs(ci):
                qa, qb = CHUNKS[ci]
                cc_inst = nc.gpsimd.collective_compute(
                    "ReduceScatter",
                    mybir.AluOpType.add,
                    replica_groups=[list(range(N_CORES))],
                    ins=[y_acc.ap()[qa:qb].opt()],
                    outs=[rs_b.ap()[COFF[ci]:COFF[ci] + CROWS[ci], :].opt()],
                )
                for wi in range(qa, qb):
                    add_dep_helper(cc_inst.ins, yacc_wr[wi].ins,
                                   reason="rs after y_acc init")
                rs_insts[ci] = cc_inst

            # ---- emission schedule ----
            # Downs for seg0-covered chunks run right after each expert's
            # seg0 gu; the small seg1 gu and last chunks come after the
            # first-half groups so RS0 can fire early.
            seg0 = _segs(Cu[0])
            seg1 = _segs(Cu[1])
            done_j = set(j for j, r in enumerate(ovl) if r is None)
            next_tc = 0
            n_sh = 0

            rs_bounds = {CHUNKS[ci][1] * WG: ci for ci in range(len(CHUNKS))}

            def flush_groups():
                nonlocal next_tc
                while (next_tc < NTC
                       and next_tc * 128 < n_sh * 512
                       and all(j in done_j for _, j in contrib[next_tc])):
                    emit_group(next_tc)
                    next_tc += 1
                    if next_tc % WG == 0:
                        emit_yacc_write(next_tc // WG - 1)
                    if next_tc in rs_bounds:
                        emit_rs(rs_bounds[next_tc])

            # First-half groups (tc < 8) only need shared segs 0-1 and
            # seg0-covered chunks, so RS0 fires before sh2/sh3/seg1 run.
            early = [min(4, NCC[e]) for e in range(EPC)]
            emit_shared_gu(0)
            emit_shared_gu(1)
            n_sh = 2
            emit_expert_gu(0, seg0[0])
            for cc in range(early[0]):
                emit_down(0, cc)
                done_j.add(cc)
            emit_expert_gu(1, seg1[0])
            for cc in range(early[1]):
                emit_down(1, cc)
                done_j.add(NCC0 + cc)
            flush_groups()
            emit_shared_gu(2)
            n_sh = 3
            emit_shared_gu(3)
            n_sh = 4
            flush_groups()
            for s in seg0[1:]:
                emit_expert_gu(0, s)
            for cc in range(early[0], NCC[0]):
                emit_down(0, cc)
                done_j.add(cc)
            flush_groups()
            for s in seg1[1:]:
                emit_expert_gu(1, s)
            for cc in range(early[1], NCC[1]):
                emit_down(1, cc)
                done_j.add(NCC0 + cc)
            flush_groups()
            assert next_tc == NTC, f"groups not all emitted: {next_tc}"

            # DRAM->DRAM copy of the RS shards to the kernel output
            for ci in range(len(CHUNKS)):
                cp = nc.sync.dma_start(
                    y_out.ap()[COFF[ci]:COFF[ci] + CROWS[ci], :],
                    rs_b.ap()[COFF[ci]:COFF[ci] + CROWS[ci], :])
                add_dep_helper(cp.ins, rs_insts[ci].ins, reason="copy rs output")

    nc.compile()
    return nc


def _get_nc(Cu, Cp, ovl):
    key = (Cu, Cp, ovl)
    if key not in _nc_cache:
        _nc_cache[key] = _build(Cu, Cp, ovl)
    return _nc_cache[key]


def kernel(hidden_states, gate_w, expert_gate, expert_up, expert_down,
           shared_gate, shared_up, shared_down):
    global last_exec_time_ns
    B, S, Hh = hidden_states.shape
    x = np.asarray(hidden_states, np.float32).reshape(-1, Hh)

    # ---- host-side routing (the MoE gate) ----
    gw = np.asarray(gate_w, np.float32)
    logits = x @ gw.T
    scores = 1.0 / (1.0 + np.exp(-logits))
    order = np.argsort(-scores, axis=1, kind="stable")[:, :TOPK]
    topk_w = np.take_along_axis(scores, order, axis=1)
    topk_w = topk_w / (topk_w.sum(-1, keepdims=True) + 1e-20)
    Wc = np.zeros((T, E), np.float32)  # dense combine matrix
    np.add.at(Wc, (np.arange(T)[:, None], order), topk_w)
    sel = Wc > 0
    counts = sel.sum(0)

    # slot assignment: each core's larger expert -> slot 0
    slot_exp = []  # per core: (e_slot0, e_slot1)
    for c in range(N_CORES):
        e0, e1 = EPC * c, EPC * c + 1
        if counts[e1] > counts[e0]:
            e0, e1 = e1, e0
        slot_exp.append((e0, e1))
    Cu, Cp = [], []
    for k in range(EPC):
        m = max(int(counts[slot_exp[c][k]]) for c in range(N_CORES))
        cu = min(max(64, -(-m // 64) * 64), T)
        Cu.append(cu)
        Cp.append(-(-cu // 128) * 128)
    Cu, Cp = tuple(Cu), tuple(Cp)
    NCC = [Cp[0] // 128, Cp[1] // 128]
    NCC0 = NCC[0]

    gidx_all = np.zeros((E, Cp[0]), np.int32)
    sidx_all = np.full((E, Cp[0]), OOB, np.int32)
    for e in range(E):
        lst = np.nonzero(sel[:, e])[0].astype(np.int32)
        gidx_all[e, :len(lst)] = lst
        sidx_all[e, :len(lst)] = lst

    # ---- overlap structure: token-tile range per (slot, chunk), union ----
    ovl = []
    for k in range(EPC):
        for cc in range(NCC0):
            lo, hi = NTC, -1
            if cc < NCC[k]:
                for c in range(N_CORES):
                    e = slot_exp[c][k]
                    r = sidx_all[e, cc * 128:(cc + 1) * 128]
                    r = r[r < OOB]
                    if len(r):
                        lo = min(lo, int(r.min()) // 128)
                        hi = max(hi, int(r.max()) // 128)
            ovl.append(None if hi < 0 else (lo, hi))
    ovl = tuple(ovl)
    smap = {}
    NS = 0
    for j, r in enumerate(ovl):
        if r is None:
            continue
        for tcv in range(r[0], r[1] + 1):
            smap[(j, tcv)] = NS
            NS += 1

    # ---- cast / pack per-core inputs (the all-to-all token dispatch) ----
    x16 = x.astype(np.float16)
    xTp = np.ascontiguousarray(
        x16.reshape(4, T // 4, H // 128, 128).transpose(0, 3, 2, 1))
    eg = np.asarray(expert_gate, np.float32).astype(np.float16)
    eu = np.asarray(expert_up, np.float32).astype(np.float16)
    ed = np.asarray(expert_down, np.float32).astype(np.float16)
    sg = np.asarray(shared_gate, np.float32).astype(np.float16)
    su = np.asarray(shared_up, np.float32).astype(np.float16)
    sd = np.asarray(shared_down, np.float32).astype(np.float16)

    in_maps = []
    for c in range(N_CORES):
        ex = slot_exp[c]
        xgT = np.stack([
            np.ascontiguousarray(
                x16[gidx_all[e]].T.reshape(H // 128, 128, NCC0, 128)
                .transpose(2, 1, 0, 3))
            for e in ex
        ])
        wgu = np.stack([
            np.concatenate([eg[e], eu[e]], axis=1)
            .reshape(H // 128, 128, 2 * I).transpose(1, 0, 2)
            for e in ex
        ])
        wd = np.stack([
            ed[e].reshape(I // 128, 128, H).transpose(1, 0, 2)
            for e in ex
        ])
        sgsu = np.concatenate([sg[:, c * SIC:(c + 1) * SIC],
                               su[:, c * SIC:(c + 1) * SIC]], axis=1)
        sgsup = sgsu.reshape(H // 128, 128, 2 * SIC).transpose(1, 0, 2)
        # S-tiles: selection matrices with combine weights folded in
        Sp = np.zeros((128, max(NS, 1), 128), np.float16)
        for k, e in enumerate(ex):
            for cc in range(NCC[k]):
                j = k * NCC0 + cc
                if ovl[j] is None:
                    continue
                toks = sidx_all[e, cc * 128:(cc + 1) * 128]
                valid = toks < OOB
                wv = Wc[gidx_all[e, cc * 128:(cc + 1) * 128], e] * valid
                for tcv in range(ovl[j][0], ovl[j][1] + 1):
                    n = smap[(j, tcv)]
                    m = valid & (toks // 128 == tcv)
                    pp = np.nonzero(m)[0]
                    Sp[pp, n, toks[m] % 128] = wv[pp].astype(np.float16)
        in_maps.append({
            "xTp": xTp,
            "xgT16": xgT,
            "wgup": np.ascontiguousarray(wgu),
            "wdp": np.ascontiguousarray(wd),
            "sgsup": np.ascontiguousarray(sgsup),
            "sd16": np.ascontiguousarray(sd[c * SIC:(c + 1) * SIC, :]),
            "Sp": Sp,
        })

    nc = _get_nc(Cu, Cp, ovl)
    trace = bool(int(os.environ.get("KERNEL_TRACE", "0")))
    res = run_bass_kernel_spmd(
        nc, in_maps, core_ids=list(range(N_CORES)), trace=trace
    )
    last_exec_time_ns = res.exec_time_ns

    # reassemble: RS chunk ci covers quarters [qa, qb); shard c is the
    # flat 1/8 slice: row r -> A = c*rows + r, tci = A % 4, B = A // 4,
    # q = qa + B // 128, p = B % 128, token = q*512 + tci*128 + p
    out = np.empty((T, Hh), np.float32)
    for c in range(N_CORES):
        yo = np.asarray(res.results[c]["y_out"], np.float32)
        for ci, (qa, qb) in enumerate(CHUNKS):
            rows = CROWS[ci]
            A = c * rows + np.arange(rows)
            tci = A % 4
            Bq = A // 4
            tok = (qa + Bq // 128) * 512 + tci * 128 + (Bq % 128)
            out[tok] = yo[COFF[ci]:COFF[ci] + rows]
    return out.reshape(B, S, Hh).astype(np.float32)


# revision 37
# speedup vs baseline: 1.1091x; 1.1007x over previous
"""DeepseekV3 MoE layer on 8 Trainium2 NeuronCores.

Strategy (expert-parallel, per sharding hint):
- Host does the routing (gate scores, top-4, combine weights) and the
  all-to-all token dispatch as input sharding: each core receives its 2
  experts' gathered tokens pre-transposed to [H, C] fp16. Each core's
  LARGER expert goes in slot 0 so slot 1 compiles with a smaller padded
  token count (less PE waste).
- The gathered->dense combine is a MATMUL against host-built selection
  matrices (S-tiles) with the combine weights folded in: for each output
  token-tile, one PSUM accumulation group sums the shared-expert down
  projection and both experts' contributions (S^T @ z). No indirect
  DMAs / scatter-adds anywhere.
- All large inputs are host-packed so each DMA is 128 long contiguous
  descriptors (HWDGE issue cost scales with descriptor count).
- A tiny dummy collective at t~0 absorbs the ~11.5us first-collective
  entry cost; NHALF chunked ReduceScatters then overlap compute.
"""

import os
import sys
import types

sys.path.insert(0, "/opt/trn_rl_repo")

# antenv.axon_hooks shim so trace=True works under axon (profiling only).
if "antenv.axon_hooks" not in sys.modules:
    _hook_holder = [None]
    _hooks_mod = types.ModuleType("antenv.axon_hooks")
    _hooks_mod.set_axon_ntff_profile_hook = lambda h: _hook_holder.__setitem__(0, h)
    _hooks_mod.get_axon_ntff_profile_hook = lambda: _hook_holder[0]
    sys.modules["antenv.axon_hooks"] = _hooks_mod
    try:
        from trn_agent_boot.trn_boot import _ntff_profile_via_ctypes

        _hook_holder[0] = _ntff_profile_via_ctypes("/opt/axon/libaxon_pjrt.so")
    except Exception:
        pass

import numpy as np

import concourse.mybir as mybir
from concourse import bacc
from concourse.tile import TileContext, add_dep_helper
from concourse.bass_utils import run_bass_kernel_spmd

N_CORES = 8
T, H, E, I = 2048, 1024, 16, 512
TOPK = 4
SIC = 128  # shared-expert intermediate slice per core (1024 / 8)
EPC = 2  # experts per core
OOB = 1 << 20
NTC = T // 128  # output token tiles
NQ = 4  # y_acc quarters (4 tiles each)
# RS chunks as quarter ranges (2MB chunks; smaller ones run below
# 60 GB/s and lose more than the extra pipelining gains)
CHUNKS = [(0, 2), (2, 4)]
CROWS = [(q1 - q0) * T // NQ // N_CORES for q0, q1 in CHUNKS]  # [128, 64, 64]
COFF = [sum(CROWS[:i]) for i in range(len(CHUNKS))]

F16 = mybir.dt.float16
F32 = mybir.dt.float32
AF = mybir.ActivationFunctionType

_nc_cache = {}
last_exec_time_ns = None


def _segs(c_use):
    out = []
    s0 = 0
    while s0 < c_use:
        s1 = min(s0 + 512, c_use)
        out.append((s0, s1))
        s0 = s1
    return out


def _build(Cu, Cp, ovl):
    """Cu/Cp: per-slot (use, pad) token counts. ovl: tuple over
    j=(k*NCC0+cc) of (tc_lo, tc_hi) token-tile range (union over cores),
    None for absent chunks."""
    NCC = [Cp[0] // 128, Cp[1] // 128]
    NCC0 = NCC[0]
    smap = {}
    NS = 0
    for j, r in enumerate(ovl):
        if r is None:
            continue
        for tcv in range(r[0], r[1] + 1):
            smap[(j, tcv)] = NS
            NS += 1
    contrib = [[] for _ in range(NTC)]
    for (j, tcv), n in smap.items():
        contrib[tcv].append((n, j))
    for lst in contrib:
        lst.sort()

    nc = bacc.Bacc(trn_type="TRN2", target_bir_lowering=False, num_devices=N_CORES)

    # ---- I/O (host-packed for contiguous per-partition DMA) ----
    xTp = nc.dram_tensor("xTp", [4, 128, H // 128, T // 4], F16, kind="ExternalInput")
    xgT16 = nc.dram_tensor("xgT16", [EPC, NCC0, 128, H // 128, 128], F16, kind="ExternalInput")
    wgup = nc.dram_tensor("wgup", [EPC, 128, H // 128, 2 * I], F16, kind="ExternalInput")
    wdp = nc.dram_tensor("wdp", [EPC, 128, I // 128, H], F16, kind="ExternalInput")
    sgsup = nc.dram_tensor("sgsup", [128, H // 128, 2 * SIC], F16, kind="ExternalInput")
    sd16 = nc.dram_tensor("sd16", [SIC, H], F16, kind="ExternalInput")
    Sp = nc.dram_tensor("Sp", [128, max(NS, 1), 128], F16, kind="ExternalInput")

    # y_acc in quarter-major / partition-major layout: token t lives at
    # [t // 512, t % 128, (t % 512) // 128, :] so the ys->y_acc DMA is one
    # contiguous run per partition. RS shards by flat order — host
    # reassembles per chunk.
    y_acc = nc.dram_tensor("y_acc", [NQ, 128, NTC // NQ, H], F16)
    rs_b = nc.dram_tensor("rs_b", [T // N_CORES, H], F16)
    y_out = nc.dram_tensor("y_out", [T // N_CORES, H], F16, kind="ExternalOutput")
    warm_i = nc.dram_tensor("warm_i", [N_CORES, 64], F16)
    warm_o = nc.dram_tensor("warm_o", [1, 64], F16)

    with TileContext(nc) as tc:
        with (
            tc.tile_pool(name="res", bufs=1) as res,
            tc.tile_pool(name="xtq", bufs=2) as xtp_pool,
            tc.tile_pool(name="sc", bufs=3) as scp,
            tc.tile_pool(name="ps_gu", bufs=2, space="PSUM") as ps_gu,
            tc.tile_pool(name="ps_z", bufs=2, space="PSUM") as ps_z,
        ):
            # ---- resident tiles ----
            xgT_sb = res.tile([128, EPC, H // 128, Cp[0]], F16, tag="xgT")
            wgu_sb = res.tile([128, EPC, H // 128, 2 * I], F16, tag="wgu")
            wd_sb = res.tile([128, EPC, I // 128, H], F16, tag="wd")
            sgsu_sb = res.tile([128, H // 128, 2 * SIC], F16, tag="sgsu")
            sd_sb = res.tile([128, H], F16, tag="sd")
            S_sb = res.tile([128, max(NS, 1), 128], F16, tag="S")
            spT_sb = res.tile([128, T], F16, tag="spT")
            p_sb = res.tile([128, EPC, I // 128, Cp[0]], F16, tag="p")
            z_sb = res.tile([128, EPC, NCC0, H], F16, tag="z")
            ys_sb = res.tile([128, NTC, H], F16, tag="ys")

            xt_tiles = []

            # warm up the collectives engine (absorbs first-CC entry cost)
            nc.gpsimd.collective_compute(
                "ReduceScatter", mybir.AluOpType.add,
                replica_groups=[list(range(N_CORES))],
                ins=[warm_i.ap().opt()], outs=[warm_o.ap().opt()])

            # ---- preload ----
            # scalar gets sgsu (small, needed first) so it lands while
            # sync streams the first xT quarter (split for earlier start)
            nc.scalar.dma_start(sgsu_sb[:], sgsup.ap())
            for q in range(4):
                xtq = xtp_pool.tile([128, H // 128, T // 4], F16, tag="xtq")
                if q == 0:
                    nc.sync.dma_start(xtq[:, 0:4, :], xTp.ap()[0, :, 0:4, :])
                    nc.sync.dma_start(xtq[:, 4:8, :], xTp.ap()[0, :, 4:8, :])
                else:
                    nc.sync.dma_start(xtq[:], xTp.ap()[q])
                xt_tiles.append(xtq)
            # scalar (HWDGE): sd + packed expert weights, consumption order
            nc.scalar.dma_start(sd_sb[:], sd16.ap())
            for e in range(EPC):
                nc.scalar.dma_start(wgu_sb[:, e], wgup.ap()[e])
            for e in range(EPC):
                nc.scalar.dma_start(wd_sb[:, e], wdp.ap()[e])
            # gpsimd (SWDGE): gathered tokens + S-tiles
            for e in range(EPC):
                for cc in range(NCC[e]):
                    nc.gpsimd.dma_start(
                        xgT_sb[:, e, :, cc * 128:(cc + 1) * 128], xgT16.ap()[e, cc])
            nc.gpsimd.dma_start(S_sb[:], Sp.ap())

            # zero the pad columns of p (read by down-matmul lhsT chunks)
            for e in range(EPC):
                if Cp[e] > Cu[e]:
                    nc.vector.memset(p_sb[:, e, :, Cu[e]:Cp[e]], 0)

            # ---- emit helpers ----
            def emit_shared_gu(s):
                pg = ps_gu.tile([128, 512], F32, tag="pg")
                pu = ps_gu.tile([128, 512], F32, tag="pu")
                for ho in range(H // 128):
                    nc.tensor.matmul(
                        pg[:], lhsT=sgsu_sb[:, ho, 0:SIC], rhs=xt_tiles[s][:, ho, :],
                        start=(ho == 0), stop=(ho == H // 128 - 1))
                    nc.tensor.matmul(
                        pu[:], lhsT=sgsu_sb[:, ho, SIC:2 * SIC], rhs=xt_tiles[s][:, ho, :],
                        start=(ho == 0), stop=(ho == H // 128 - 1))
                sg = scp.tile([128, 512], F16, tag="sg")
                nc.scalar.activation(sg[:], pg[:], AF.Silu)
                nc.vector.tensor_tensor(
                    out=spT_sb[:, s * 512:(s + 1) * 512], in0=sg[:], in1=pu[:],
                    op=mybir.AluOpType.mult)

            def emit_expert_gu(e, seg):
                a, b = seg
                w = b - a
                for it in range(I // 128):
                    pg_full = ps_gu.tile([128, 512], F32, tag="pg")
                    pg = pg_full[:, :w]
                    pu_full = ps_gu.tile([128, 512], F32, tag="pu")
                    pu = pu_full[:, :w]
                    for ho in range(H // 128):
                        nc.tensor.matmul(
                            pg[:], lhsT=wgu_sb[:, e, ho, it * 128:(it + 1) * 128],
                            rhs=xgT_sb[:, e, ho, a:b],
                            start=(ho == 0), stop=(ho == H // 128 - 1))
                        nc.tensor.matmul(
                            pu[:], lhsT=wgu_sb[:, e, ho, I + it * 128:I + (it + 1) * 128],
                            rhs=xgT_sb[:, e, ho, a:b],
                            start=(ho == 0), stop=(ho == H // 128 - 1))
                    sg_full = scp.tile([128, 512], F16, tag="sg")
                    sg = sg_full[:, :w]
                    nc.scalar.activation(sg[:], pg[:], AF.Silu)
                    nc.vector.tensor_tensor(
                        out=p_sb[:, e, it, a:b], in0=sg[:], in1=pu[:],
                        op=mybir.AluOpType.mult)

            def emit_down(e, cc):
                pz = ps_z.tile([128, H], F32, tag="pz")
                for it in range(I // 128):
                    for hf in range(2):
                        nc.tensor.matmul(
                            pz[:, hf * 512:(hf + 1) * 512],
                            lhsT=p_sb[:, e, it, cc * 128:(cc + 1) * 128],
                            rhs=wd_sb[:, e, it, hf * 512:(hf + 1) * 512],
                            start=(it == 0), stop=(it == I // 128 - 1))
                nc.vector.tensor_copy(z_sb[:, e, cc, :], pz[:])

            def emit_group(tc_i):
                """shared down + S-combine for output token tile tc_i.
                Reuses the gu-phase PSUM banks (pg/pu tags)."""
                py0 = ps_gu.tile([128, 512], F32, tag="pg")
                py1 = ps_gu.tile([128, 512], F32, tag="pu")
                nmm = len(contrib[tc_i]) + 1
                for hf, py in enumerate((py0, py1)):
                    nc.tensor.matmul(
                        py[:],
                        lhsT=spT_sb[:, tc_i * 128:(tc_i + 1) * 128],
                        rhs=sd_sb[:, hf * 512:(hf + 1) * 512],
                        start=True, stop=(nmm == 1))
                    for i, (n, j) in enumerate(contrib[tc_i]):
                        e, cc = j // NCC0, j % NCC0
                        nc.tensor.matmul(
                            py[:],
                            lhsT=S_sb[:, n, :],
                            rhs=z_sb[:, e, cc, hf * 512:(hf + 1) * 512],
                            start=False, stop=(i == nmm - 2))
                    if hf == 0:
                        nc.scalar.activation(
                            ys_sb[:, tc_i, 0:512], py[:], AF.Copy)
                    else:
                        nc.vector.tensor_copy(ys_sb[:, tc_i, 512:1024], py[:])

            WG = NTC // NQ  # output tiles per y_acc write (one quarter)
            yacc_wr = []  # (t0, t1, inst)
            rs_insts = [None] * len(CHUNKS)

            def emit_yacc_write(t0, t1):
                # tiles [t0, t1), all within quarter t0 // WG
                q, off = t0 // WG, t0 % WG
                w = nc.sync.dma_start(
                    y_acc.ap()[q, :, off:off + (t1 - t0), :],
                    ys_sb[:, t0:t1, :])
                yacc_wr.append((t0, t1, w))

            def emit_rs(ci):
                qa, qb = CHUNKS[ci]
                cc_inst = nc.gpsimd.collective_compute(
                    "ReduceScatter",
                    mybir.AluOpType.add,
                    replica_groups=[list(range(N_CORES))],
                    ins=[y_acc.ap()[qa:qb].opt()],
                    outs=[rs_b.ap()[COFF[ci]:COFF[ci] + CROWS[ci], :].opt()],
                )
                for t0, t1, w in yacc_wr:
                    if t0 < qb * WG and t1 > qa * WG:
                        add_dep_helper(cc_inst.ins, w.ins,
                                       reason="rs after y_acc init")
                rs_insts[ci] = cc_inst

            # ---- emission schedule ----
            # Downs for seg0-covered chunks run right after each expert's
            # seg0 gu; the small seg1 gu and last chunks come after the
            # first-half groups so RS0 can fire early.
            seg0 = _segs(Cu[0])
            seg1 = _segs(Cu[1])
            done_j = set(j for j, r in enumerate(ovl) if r is None)
            next_tc = 0
            n_sh = 0

            rs_bounds = {CHUNKS[ci][1] * WG: ci for ci in range(len(CHUNKS))}

            def flush_groups():
                nonlocal next_tc
                while (next_tc < NTC
                       and next_tc * 128 < n_sh * 512
                       and all(j in done_j for _, j in contrib[next_tc])):
                    emit_group(next_tc)
                    next_tc += 1
                    # per-quarter writes; per-tile in the last quarter so
                    # the final RS trigger waits only a 0.25MB write
                    if next_tc > NTC - WG:
                        emit_yacc_write(next_tc - 1, next_tc)
                    elif next_tc % WG == 0:
                        emit_yacc_write(next_tc - WG, next_tc)
                    if next_tc in rs_bounds:
                        emit_rs(rs_bounds[next_tc])

            # First-half groups (tc < 8) only need shared segs 0-1 and
            # seg0-covered chunks, so RS0 fires before sh2/sh3/seg1 run.
            early = [min(4, NCC[e]) for e in range(EPC)]
            emit_shared_gu(0)
            emit_shared_gu(1)
            n_sh = 2
            emit_expert_gu(0, seg0[0])
            for cc in range(early[0]):
                emit_down(0, cc)
                done_j.add(cc)
            emit_expert_gu(1, seg1[0])
            for cc in range(early[1]):
                emit_down(1, cc)
                done_j.add(NCC0 + cc)
            flush_groups()
            emit_shared_gu(2)
            n_sh = 3
            emit_shared_gu(3)
            n_sh = 4
            flush_groups()
            for s in seg0[1:]:
                emit_expert_gu(0, s)
            for cc in range(early[0], NCC[0]):
                emit_down(0, cc)
                done_j.add(cc)
            flush_groups()
            for s in seg1[1:]:
                emit_expert_gu(1, s)
            for cc in range(early[1], NCC[1]):
                emit_down(1, cc)
                done_j.add(NCC0 + cc)
            flush_groups()
            assert next_tc == NTC, f"groups not all emitted: {next_tc}"

            # DRAM->DRAM copy of the RS shards to the kernel output
            for ci in range(len(CHUNKS)):
                cp = nc.sync.dma_start(
                    y_out.ap()[COFF[ci]:COFF[ci] + CROWS[ci], :],
                    rs_b.ap()[COFF[ci]:COFF[ci] + CROWS[ci], :])
                add_dep_helper(cp.ins, rs_insts[ci].ins, reason="copy rs output")

    nc.compile()
    return nc


def _get_nc(Cu, Cp, ovl):
    key = (Cu, Cp, ovl)
    if key not in _nc_cache:
        _nc_cache[key] = _build(Cu, Cp, ovl)
    return _nc_cache[key]


def kernel(hidden_states, gate_w, expert_gate, expert_up, expert_down,
           shared_gate, shared_up, shared_down):
    global last_exec_time_ns
    B, S, Hh = hidden_states.shape
    x = np.asarray(hidden_states, np.float32).reshape(-1, Hh)

    # ---- host-side routing (the MoE gate) ----
    gw = np.asarray(gate_w, np.float32)
    logits = x @ gw.T
    scores = 1.0 / (1.0 + np.exp(-logits))
    order = np.argsort(-scores, axis=1, kind="stable")[:, :TOPK]
    topk_w = np.take_along_axis(scores, order, axis=1)
    topk_w = topk_w / (topk_w.sum(-1, keepdims=True) + 1e-20)
    Wc = np.zeros((T, E), np.float32)  # dense combine matrix
    np.add.at(Wc, (np.arange(T)[:, None], order), topk_w)
    sel = Wc > 0
    counts = sel.sum(0)

    # slot assignment: each core's larger expert -> slot 0
    slot_exp = []  # per core: (e_slot0, e_slot1)
    for c in range(N_CORES):
        e0, e1 = EPC * c, EPC * c + 1
        if counts[e1] > counts[e0]:
            e0, e1 = e1, e0
        slot_exp.append((e0, e1))
    Cu, Cp = [], []
    for k in range(EPC):
        m = max(int(counts[slot_exp[c][k]]) for c in range(N_CORES))
        cu = min(max(64, -(-m // 64) * 64), T)
        Cu.append(cu)
        Cp.append(-(-cu // 128) * 128)
    Cu, Cp = tuple(Cu), tuple(Cp)
    NCC = [Cp[0] // 128, Cp[1] // 128]
    NCC0 = NCC[0]

    gidx_all = np.zeros((E, Cp[0]), np.int32)
    sidx_all = np.full((E, Cp[0]), OOB, np.int32)
    for e in range(E):
        lst = np.nonzero(sel[:, e])[0].astype(np.int32)
        gidx_all[e, :len(lst)] = lst
        sidx_all[e, :len(lst)] = lst

    # ---- overlap structure: token-tile range per (slot, chunk), union ----
    ovl = []
    for k in range(EPC):
        for cc in range(NCC0):
            lo, hi = NTC, -1
            if cc < NCC[k]:
                for c in range(N_CORES):
                    e = slot_exp[c][k]
                    r = sidx_all[e, cc * 128:(cc + 1) * 128]
                    r = r[r < OOB]
                    if len(r):
                        lo = min(lo, int(r.min()) // 128)
                        hi = max(hi, int(r.max()) // 128)
            ovl.append(None if hi < 0 else (lo, hi))
    ovl = tuple(ovl)
    smap = {}
    NS = 0
    for j, r in enumerate(ovl):
        if r is None:
            continue
        for tcv in range(r[0], r[1] + 1):
            smap[(j, tcv)] = NS
            NS += 1

    # ---- cast / pack per-core inputs (the all-to-all token dispatch) ----
    x16 = x.astype(np.float16)
    xTp = np.ascontiguousarray(
        x16.reshape(4, T // 4, H // 128, 128).transpose(0, 3, 2, 1))
    eg = np.asarray(expert_gate, np.float32).astype(np.float16)
    eu = np.asarray(expert_up, np.float32).astype(np.float16)
    ed = np.asarray(expert_down, np.float32).astype(np.float16)
    sg = np.asarray(shared_gate, np.float32).astype(np.float16)
    su = np.asarray(shared_up, np.float32).astype(np.float16)
    sd = np.asarray(shared_down, np.float32).astype(np.float16)

    in_maps = []
    for c in range(N_CORES):
        ex = slot_exp[c]
        xgT = np.stack([
            np.ascontiguousarray(
                x16[gidx_all[e]].T.reshape(H // 128, 128, NCC0, 128)
                .transpose(2, 1, 0, 3))
            for e in ex
        ])
        wgu = np.stack([
            np.concatenate([eg[e], eu[e]], axis=1)
            .reshape(H // 128, 128, 2 * I).transpose(1, 0, 2)
            for e in ex
        ])
        wd = np.stack([
            ed[e].reshape(I // 128, 128, H).transpose(1, 0, 2)
            for e in ex
        ])
        sgsu = np.concatenate([sg[:, c * SIC:(c + 1) * SIC],
                               su[:, c * SIC:(c + 1) * SIC]], axis=1)
        sgsup = sgsu.reshape(H // 128, 128, 2 * SIC).transpose(1, 0, 2)
        # S-tiles: selection matrices with combine weights folded in
        Sp = np.zeros((128, max(NS, 1), 128), np.float16)
        for k, e in enumerate(ex):
            for cc in range(NCC[k]):
                j = k * NCC0 + cc
                if ovl[j] is None:
                    continue
                toks = sidx_all[e, cc * 128:(cc + 1) * 128]
                valid = toks < OOB
                wv = Wc[gidx_all[e, cc * 128:(cc + 1) * 128], e] * valid
                for tcv in range(ovl[j][0], ovl[j][1] + 1):
                    n = smap[(j, tcv)]
                    m = valid & (toks // 128 == tcv)
                    pp = np.nonzero(m)[0]
                    Sp[pp, n, toks[m] % 128] = wv[pp].astype(np.float16)
        in_maps.append({
            "xTp": xTp,
            "xgT16": xgT,
            "wgup": np.ascontiguousarray(wgu),
            "wdp": np.ascontiguousarray(wd),
            "sgsup": np.ascontiguousarray(sgsup),
            "sd16": np.ascontiguousarray(sd[c * SIC:(c + 1) * SIC, :]),
            "Sp": Sp,
        })

    nc = _get_nc(Cu, Cp, ovl)
    trace = bool(int(os.environ.get("KERNEL_TRACE", "0")))
    res = run_bass_kernel_spmd(
        nc, in_maps, core_ids=list(range(N_CORES)), trace=trace
    )
    last_exec_time_ns = res.exec_time_ns

    # reassemble: RS chunk ci covers quarters [qa, qb); shard c is the
    # flat 1/8 slice: row r -> A = c*rows + r, tci = A % 4, B = A // 4,
    # q = qa + B // 128, p = B % 128, token = q*512 + tci*128 + p
    out = np.empty((T, Hh), np.float32)
    for c in range(N_CORES):
        yo = np.asarray(res.results[c]["y_out"], np.float32)
        for ci, (qa, qb) in enumerate(CHUNKS):
            rows = CROWS[ci]
            A = c * rows + np.arange(rows)
            tci = A % 4
            Bq = A // 4
            tok = (qa + Bq // 128) * 512 + tci * 128 + (Bq % 128)
            out[tok] = yo[COFF[ci]:COFF[ci] + rows]
    return out.reshape(B, S, Hh).astype(np.float32)
